# revision 15
# baseline (speedup 1.0000x reference)
"""Causal depthwise conv1d (B=8, S=4096, H=2048, KS=4) on 8 trn2 NeuronCores.

Strategy:
  - Shard batch across the 8 cores (one batch element each, no halo needed).
  - Host-side transpose each batch element to (H, S) so the device sees
    channels on SBUF partitions and the sequence dim contiguous on the free
    axis: fully coalesced f32 DMA both ways, conv shifts become free-dim AP
    offsets, and the per-channel weights become per-partition [P,1] scalars.
  - Per 128-channel block: two ACT passes (w3*x + bias, w2*x_{-1}) and three
    DVE passes (two fused scalar_tensor_tensor multiply-adds + one add).
"""

import numpy as np

B, S, H, KS = 8, 4096, 2048, 4
NCORES = 8
PB = 128            # SBUF partitions
HB = H // PB        # 16 channel blocks per core
PAD = 4             # left zero-pad columns in the x tile (3 used + 1 align)

# test.py can inject e.g. {"trace": True, "tmpdir": ...} here.
RUN_KWARGS = {}
LAST_RESULTS = []   # last BassKernelResults, for the harness to inspect

_cached = {}


def _build():
    import concourse.bacc as bacc
    import concourse.mybir as mybir
    import concourse.tile as tile

    f32 = mybir.dt.float32
    Alu = mybir.AluOpType
    Act = mybir.ActivationFunctionType

    nc = bacc.Bacc(
        "TRN2",
        target_bir_lowering=False,
        debug=False,
        num_devices=NCORES,
    )
    xT = nc.dram_tensor("xT", [H, S], f32, kind="ExternalInput")
    wp = nc.dram_tensor("wp", [PB, HB * 5], f32, kind="ExternalInput")
    yT = nc.dram_tensor("yT", [H, S], f32, kind="ExternalOutput")

    SC = S // 2  # load-split width
    with tile.TileContext(nc) as tc:
        with tc.tile_pool(name="wpool", bufs=1) as wpool, \
             tc.tile_pool(name="xpool", bufs=3) as xpool, \
             tc.tile_pool(name="data", bufs=4) as pool:
            wsb = wpool.tile([PB, HB * 5], f32)
            nc.sync.dma_start(wsb[:], wp[:])
            # Tiny no-dep ACTIVATE so the ACT table load overlaps the first
            # x DMA instead of serializing in front of the first product.
            warm = wpool.tile([PB, 2], f32)
            nc.vector.memset(warm[:], 0.0)
            nc.scalar.activation(warm[:], warm[:], Act.Identity, bias=0.0,
                                 scale=1.0)
            for hb in range(HB):
                rows = slice(hb * PB, (hb + 1) * PB)
                xt = xpool.tile([PB, PAD + S], f32)
                nc.vector.memset(xt[:, 0:PAD], 0.0)
                c = hb * 5
                w0 = wsb[:, c + 0:c + 1]
                w1 = wsb[:, c + 1:c + 2]
                w2 = wsb[:, c + 2:c + 3]
                w3 = wsb[:, c + 3:c + 4]
                bb = wsb[:, c + 4:c + 5]

                # First tile: fine chunks so compute starts ~3us after t0.
                # Last tile: split so the final store is half-size (shorter
                # tail). Middle tiles: full-width ops (min DVE overhead).
                if hb == 0:
                    chunks = [S // 8] * 2 + [S // 4] + [S // 2]
                elif hb == HB - 1:
                    chunks = [S // 2, S // 4, S // 4]
                else:
                    chunks = [S]
                s0 = 0
                for ci, cw in enumerate(chunks):
                    base = PAD + s0
                    nc.sync.dma_start(xt[:, base:base + cw],
                                      xT[rows, s0:s0 + cw])
                    t3 = pool.tile([PB, S], f32, tag="t3")
                    t2 = pool.tile([PB, S], f32, tag="t2")
                    if hb == 0 and ci == 0:
                        # products on DVE (2x-mode tensor_scalar) so the very
                        # first compute has no ACT table-load/product in its
                        # critical path
                        nc.vector.tensor_scalar(t3[:, :cw], xt[:, base:base + cw],
                                                w3, bb, op0=Alu.mult,
                                                op1=Alu.add)
                        nc.vector.tensor_scalar(t2[:, :cw],
                                                xt[:, base - 1:base - 1 + cw],
                                                w2, None, op0=Alu.mult)
                    else:
                        # t3 = w3 * x[s] + bias
                        nc.scalar.activation(t3[:, :cw], xt[:, base:base + cw],
                                             Act.Identity, bias=bb, scale=w3)
                        # t2 = w2 * x[s-1]
                        nc.scalar.activation(t2[:, :cw],
                                             xt[:, base - 1:base - 1 + cw],
                                             Act.Copy, scale=w2)
                    # t3 += w0 * x[s-3]
                    nc.vector.scalar_tensor_tensor(
                        t3[:, :cw], xt[:, base - 3:base - 3 + cw], w0,
                        t3[:, :cw], op0=Alu.mult, op1=Alu.add)
                    # t2 += w1 * x[s-2]
                    nc.vector.scalar_tensor_tensor(
                        t2[:, :cw], xt[:, base - 2:base - 2 + cw], w1,
                        t2[:, :cw], op0=Alu.mult, op1=Alu.add)
                    # t3 = t3 + t2, stored from the same tile
                    nc.vector.tensor_tensor(t3[:, :cw], t3[:, :cw], t2[:, :cw],
                                            op=Alu.add)
                    nc.scalar.dma_start(yT[rows, s0:s0 + cw], t3[:, :cw])
                    s0 += cw
    nc.compile()
    return nc


def get_nc():
    if "nc" not in _cached:
        _cached["nc"] = _build()
    return _cached["nc"]


def pack_weights(weight, bias):
    wp = np.empty((PB, HB * 5), dtype=np.float32)
    for hb in range(HB):
        sl = slice(hb * PB, (hb + 1) * PB)
        for k in range(KS):
            wp[:, hb * 5 + k] = weight[k, sl]
        wp[:, hb * 5 + 4] = bias[sl]
    return wp


def kernel(x, weight, bias):
    from concourse.bass_utils import run_bass_kernel_spmd

    x = np.ascontiguousarray(np.asarray(x, dtype=np.float32))
    weight = np.asarray(weight, dtype=np.float32)
    bias = np.asarray(bias, dtype=np.float32)
    assert x.shape == (B, S, H), x.shape
    assert weight.shape == (KS, H), weight.shape
    assert bias.shape == (H,), bias.shape

    nc = get_nc()
    wp = pack_weights(weight, bias)
    xT = np.ascontiguousarray(x.transpose(0, 2, 1))  # (B, H, S)

    in_maps = [{"xT": xT[i], "wp": wp} for i in range(NCORES)]
    try:
        res = run_bass_kernel_spmd(nc, in_maps, core_ids=list(range(NCORES)),
                                   **RUN_KWARGS)
    except Exception:
        # one retry for transient device hiccups
        res = run_bass_kernel_spmd(nc, in_maps, core_ids=list(range(NCORES)),
                                   **RUN_KWARGS)
    LAST_RESULTS.clear()
    LAST_RESULTS.append(res)
    y = np.stack([res.results[i]["yT"] for i in range(NCORES)])  # (B, H, S)
    return np.ascontiguousarray(y.transpose(0, 2, 1))


# revision 21
# speedup vs baseline: 1.0145x; 1.0145x over previous
"""Causal depthwise conv1d (B=8, S=4096, H=2048, KS=4) on 8 trn2 NeuronCores.

Strategy:
  - Shard batch across the 8 cores (one batch element each, no halo needed).
  - Host-side transpose each batch element to (H, S) so the device sees
    channels on SBUF partitions and the sequence dim contiguous on the free
    axis: fully coalesced f32 DMA both ways, conv shifts become free-dim AP
    offsets, and the per-channel weights become per-partition [P,1] scalars.
  - Per 128-channel block: two ACT passes (w3*x + bias, w2*x_{-1}) and three
    DVE passes (two fused scalar_tensor_tensor multiply-adds + one add).
"""

import numpy as np

B, S, H, KS = 8, 4096, 2048, 4
NCORES = 8
PB = 128            # SBUF partitions
HB = H // PB        # 16 channel blocks per core
PAD = 4             # left zero-pad columns in the x tile (3 used + 1 align)

# test.py can inject e.g. {"trace": True, "tmpdir": ...} here.
RUN_KWARGS = {}
LAST_RESULTS = []   # last BassKernelResults, for the harness to inspect

_cached = {}


def _build():
    import concourse.bacc as bacc
    import concourse.mybir as mybir
    import concourse.tile as tile

    f32 = mybir.dt.float32
    Alu = mybir.AluOpType
    Act = mybir.ActivationFunctionType

    nc = bacc.Bacc(
        "TRN2",
        target_bir_lowering=False,
        debug=False,
        num_devices=NCORES,
    )
    xT = nc.dram_tensor("xT", [H, S], f32, kind="ExternalInput")
    wp = nc.dram_tensor("wp", [PB, HB * 5], f32, kind="ExternalInput")
    yT = nc.dram_tensor("yT", [H, S], f32, kind="ExternalOutput")

    SC = S // 2  # load-split width
    with tile.TileContext(nc) as tc:
        with tc.tile_pool(name="wpool", bufs=1) as wpool, \
             tc.tile_pool(name="xpool", bufs=3) as xpool, \
             tc.tile_pool(name="data", bufs=4) as pool:
            wsb = wpool.tile([PB, HB * 5], f32)
            # scalar ring: keeps the sync-ring FIFO clear for the first x load
            nc.scalar.dma_start(wsb[:], wp[:])
            # Tiny no-dep ACTIVATE so the ACT table load overlaps the first
            # x DMA instead of serializing in front of the first product.
            warm = wpool.tile([PB, 2], f32)
            nc.vector.memset(warm[:], 0.0)
            nc.scalar.activation(warm[:], warm[:], Act.Identity, bias=0.0,
                                 scale=1.0)
            for hb in range(HB):
                rows = slice(hb * PB, (hb + 1) * PB)
                xt = xpool.tile([PB, PAD + S], f32)
                nc.vector.memset(xt[:, 0:PAD], 0.0)
                c = hb * 5
                w0 = wsb[:, c + 0:c + 1]
                w1 = wsb[:, c + 1:c + 2]
                w2 = wsb[:, c + 2:c + 3]
                w3 = wsb[:, c + 3:c + 4]
                bb = wsb[:, c + 4:c + 5]

                # First tile: fine chunks so compute starts ~3us after t0.
                # Last tile: split so the final store is half-size (shorter
                # tail). Middle tiles: full-width ops (min DVE overhead).
                if hb == 0:
                    chunks = [S // 8] * 2 + [S // 4] + [S // 2]
                elif hb == HB - 1:
                    chunks = [S // 2, S // 4, S // 4]
                else:
                    chunks = [S]
                s0 = 0
                for ci, cw in enumerate(chunks):
                    base = PAD + s0
                    nc.sync.dma_start(xt[:, base:base + cw],
                                      xT[rows, s0:s0 + cw])
                    t3 = pool.tile([PB, S], f32, tag="t3", bufs=5)
                    t2 = pool.tile([PB, S], f32, tag="t2", bufs=3)
                    if hb == 0 and ci == 0:
                        # products on DVE (2x-mode tensor_scalar) so the very
                        # first compute has no ACT table-load/product in its
                        # critical path
                        nc.vector.tensor_scalar(t3[:, :cw], xt[:, base:base + cw],
                                                w3, bb, op0=Alu.mult,
                                                op1=Alu.add)
                        nc.vector.tensor_scalar(t2[:, :cw],
                                                xt[:, base - 1:base - 1 + cw],
                                                w2, None, op0=Alu.mult)
                    else:
                        # t3 = w3 * x[s] + bias
                        nc.scalar.activation(t3[:, :cw], xt[:, base:base + cw],
                                             Act.Identity, bias=bb, scale=w3)
                        # t2 = w2 * x[s-1]
                        nc.scalar.activation(t2[:, :cw],
                                             xt[:, base - 1:base - 1 + cw],
                                             Act.Copy, scale=w2)
                    # t3 += w0 * x[s-3]
                    nc.vector.scalar_tensor_tensor(
                        t3[:, :cw], xt[:, base - 3:base - 3 + cw], w0,
                        t3[:, :cw], op0=Alu.mult, op1=Alu.add)
                    # t2 += w1 * x[s-2]
                    nc.vector.scalar_tensor_tensor(
                        t2[:, :cw], xt[:, base - 2:base - 2 + cw], w1,
                        t2[:, :cw], op0=Alu.mult, op1=Alu.add)
                    # t3 = t3 + t2, stored from the same tile
                    nc.vector.tensor_tensor(t3[:, :cw], t3[:, :cw], t2[:, :cw],
                                            op=Alu.add)
                    nc.scalar.dma_start(yT[rows, s0:s0 + cw], t3[:, :cw])
                    s0 += cw
    nc.compile()
    return nc


def get_nc():
    if "nc" not in _cached:
        _cached["nc"] = _build()
    return _cached["nc"]


def pack_weights(weight, bias):
    wp = np.empty((PB, HB * 5), dtype=np.float32)
    for hb in range(HB):
        sl = slice(hb * PB, (hb + 1) * PB)
        for k in range(KS):
            wp[:, hb * 5 + k] = weight[k, sl]
        wp[:, hb * 5 + 4] = bias[sl]
    return wp


def kernel(x, weight, bias):
    from concourse.bass_utils import run_bass_kernel_spmd

    x = np.ascontiguousarray(np.asarray(x, dtype=np.float32))
    weight = np.asarray(weight, dtype=np.float32)
    bias = np.asarray(bias, dtype=np.float32)
    assert x.shape == (B, S, H), x.shape
    assert weight.shape == (KS, H), weight.shape
    assert bias.shape == (H,), bias.shape

    nc = get_nc()
    wp = pack_weights(weight, bias)
    xT = np.ascontiguousarray(x.transpose(0, 2, 1))  # (B, H, S)

    in_maps = [{"xT": xT[i], "wp": wp} for i in range(NCORES)]
    try:
        res = run_bass_kernel_spmd(nc, in_maps, core_ids=list(range(NCORES)),
                                   **RUN_KWARGS)
    except Exception:
        # one retry for transient device hiccups
        res = run_bass_kernel_spmd(nc, in_maps, core_ids=list(range(NCORES)),
                                   **RUN_KWARGS)
    LAST_RESULTS.clear()
    LAST_RESULTS.append(res)
    y = np.stack([res.results[i]["yT"] for i in range(NCORES)])  # (B, H, S)
    return np.ascontiguousarray(y.transpose(0, 2, 1))


# revision 23
# speedup vs baseline: 1.0170x; 1.0025x over previous
"""Causal depthwise conv1d (B=8, S=4096, H=2048, KS=4) on 8 trn2 NeuronCores.

Strategy:
  - Shard batch across the 8 cores (one batch element each, no halo needed).
  - Host-side transpose each batch element to (H, S) so the device sees
    channels on SBUF partitions and the sequence dim contiguous on the free
    axis: fully coalesced f32 DMA both ways, conv shifts become free-dim AP
    offsets, and the per-channel weights become per-partition [P,1] scalars.
  - Per 128-channel block: two ACT passes (w3*x + bias, w2*x_{-1}) and three
    DVE passes (two fused scalar_tensor_tensor multiply-adds + one add).
"""

import numpy as np

B, S, H, KS = 8, 4096, 2048, 4
NCORES = 8
PB = 128            # SBUF partitions
HB = H // PB        # 16 channel blocks per core
PAD = 4             # left zero-pad columns in the x tile (3 used + 1 align)

# test.py can inject e.g. {"trace": True, "tmpdir": ...} here.
RUN_KWARGS = {}
LAST_RESULTS = []   # last BassKernelResults, for the harness to inspect

_cached = {}


def _build():
    import concourse.bacc as bacc
    import concourse.mybir as mybir
    import concourse.tile as tile

    f32 = mybir.dt.float32
    Alu = mybir.AluOpType
    Act = mybir.ActivationFunctionType

    nc = bacc.Bacc(
        "TRN2",
        target_bir_lowering=False,
        debug=False,
        num_devices=NCORES,
    )
    xT = nc.dram_tensor("xT", [H, S], f32, kind="ExternalInput")
    wp = nc.dram_tensor("wp", [PB, HB * 5], f32, kind="ExternalInput")
    yT = nc.dram_tensor("yT", [H, S], f32, kind="ExternalOutput")

    SC = S // 2  # load-split width
    with tile.TileContext(nc) as tc:
        with tc.tile_pool(name="wpool", bufs=1) as wpool, \
             tc.tile_pool(name="xpool", bufs=3) as xpool, \
             tc.tile_pool(name="data", bufs=4) as pool:
            wsb = wpool.tile([PB, HB * 5], f32)
            # scalar ring: keeps the sync-ring FIFO clear for the first x load
            nc.scalar.dma_start(wsb[:], wp[:])
            # Tiny no-dep ACTIVATE so the ACT table load overlaps the first
            # x DMA instead of serializing in front of the first product.
            warm = wpool.tile([PB, 2], f32)
            nc.vector.memset(warm[:], 0.0)
            nc.scalar.activation(warm[:], warm[:], Act.Identity, bias=0.0,
                                 scale=1.0)
            for hb in range(HB):
                rows = slice(hb * PB, (hb + 1) * PB)
                xt = xpool.tile([PB, PAD + S], f32)
                nc.vector.memset(xt[:, 0:PAD], 0.0)
                c = hb * 5
                w0 = wsb[:, c + 0:c + 1]
                w1 = wsb[:, c + 1:c + 2]
                w2 = wsb[:, c + 2:c + 3]
                w3 = wsb[:, c + 3:c + 4]
                bb = wsb[:, c + 4:c + 5]

                # First tile: fine chunks so compute starts ~3us after t0.
                # Last tile: split so the final store is half-size (shorter
                # tail). Middle tiles: full-width ops (min DVE overhead).
                if hb == 0:
                    chunks = [S // 8] * 2 + [S // 4] + [S // 2]
                elif hb == HB - 1:
                    chunks = [S // 2, S // 4, S // 4]
                else:
                    chunks = [S]
                s0 = 0
                for ci, cw in enumerate(chunks):
                    base = PAD + s0
                    nc.sync.dma_start(xt[:, base:base + cw],
                                      xT[rows, s0:s0 + cw])
                    t3 = pool.tile([PB, S], f32, tag="t3", bufs=5)
                    t2 = pool.tile([PB, S], f32, tag="t2", bufs=3)
                    if hb == 0 and ci == 0:
                        # products on DVE (2x-mode tensor_scalar) so the very
                        # first compute has no ACT table-load/product in its
                        # critical path
                        nc.vector.tensor_scalar(t3[:, :cw], xt[:, base:base + cw],
                                                w3, bb, op0=Alu.mult,
                                                op1=Alu.add)
                        nc.vector.tensor_scalar(t2[:, :cw],
                                                xt[:, base - 1:base - 1 + cw],
                                                w2, None, op0=Alu.mult)
                    else:
                        # t3 = w3 * x[s] + bias
                        nc.scalar.activation(t3[:, :cw], xt[:, base:base + cw],
                                             Act.Identity, bias=bb, scale=w3)
                        # t2 = w2 * x[s-1]
                        nc.scalar.activation(t2[:, :cw],
                                             xt[:, base - 1:base - 1 + cw],
                                             Act.Copy, scale=w2)
                    # t3 += w0 * x[s-3]
                    nc.vector.scalar_tensor_tensor(
                        t3[:, :cw], xt[:, base - 3:base - 3 + cw], w0,
                        t3[:, :cw], op0=Alu.mult, op1=Alu.add)
                    # t2 += w1 * x[s-2]
                    nc.vector.scalar_tensor_tensor(
                        t2[:, :cw], xt[:, base - 2:base - 2 + cw], w1,
                        t2[:, :cw], op0=Alu.mult, op1=Alu.add)
                    # t3 = t3 + t2, stored from the same tile
                    nc.vector.tensor_tensor(t3[:, :cw], t3[:, :cw], t2[:, :cw],
                                            op=Alu.add)
                    nc.scalar.dma_start(yT[rows, s0:s0 + cw], t3[:, :cw])
                    s0 += cw
    nc.compile()
    return nc


def get_nc():
    if "nc" not in _cached:
        _cached["nc"] = _build()
    return _cached["nc"]


def pack_weights(weight, bias):
    wp = np.empty((PB, HB * 5), dtype=np.float32)
    for hb in range(HB):
        sl = slice(hb * PB, (hb + 1) * PB)
        for k in range(KS):
            wp[:, hb * 5 + k] = weight[k, sl]
        wp[:, hb * 5 + 4] = bias[sl]
    return wp


def kernel(x, weight, bias):
    from concourse.bass_utils import run_bass_kernel_spmd

    x = np.ascontiguousarray(np.asarray(x, dtype=np.float32))
    weight = np.asarray(weight, dtype=np.float32)
    bias = np.asarray(bias, dtype=np.float32)
    assert x.shape == (B, S, H), x.shape
    assert weight.shape == (KS, H), weight.shape
    assert bias.shape == (H,), bias.shape

    nc = get_nc()
    wp = pack_weights(weight, bias)
    xT = np.ascontiguousarray(x.transpose(0, 2, 1))  # (B, H, S)

    in_maps = [{"xT": xT[i], "wp": wp} for i in range(NCORES)]
    try:
        res = run_bass_kernel_spmd(nc, in_maps, core_ids=list(range(NCORES)),
                                   **RUN_KWARGS)
    except Exception:
        # one retry for transient device hiccups
        res = run_bass_kernel_spmd(nc, in_maps, core_ids=list(range(NCORES)),
                                   **RUN_KWARGS)
    LAST_RESULTS.clear()
    LAST_RESULTS.append(res)
    y = np.stack([res.results[i]["yT"] for i in range(NCORES)])  # (B, H, S)
    return np.ascontiguousarray(y.transpose(0, 2, 1))
